# revision 60
# baseline (speedup 1.0000x reference)
"""MeshConvPoint Bass/Trainium2 kernel — on-chip GPSIMD gather architecture.

Problem (per mesh b of B=8, one NeuronCore each):
    nbr_mean[c,v] = (1/deg[v]) * sum_{d<deg[v]} x[c, nbr_idx[v,d]]
    out[o,v]     = sum_c W0[o,c]*x[c,v] + W1[o,c]*nbr_mean[c,v] + b[o]

Strategy (replaces the SWDGE dma_gather baseline, which was DMA-descriptor
bound at ~233us: 256B/descriptor * 22.76ns / 16 engines * ~164k edges):

  - x lives in SBUF channel-major as a quartered table [128, 2*NE] f32:
    pair A = cols 0:NE (partitions 0..63 = quarter Q0, 64..127 = Q1),
    pair B = cols NE:2NE (Q2 / Q3).  One GPSIMD ap_gather column fetches
    TWO neighbors of the same target vertex (one per table half) at
    ~1.39ns/column -- data never leaves SBUF.
  - A host-side greedy balance assigns each vertex to a quarter so that each
    target's neighbor list splits evenly across the halves of its pair
    (columns per target ~= ceil(deg/2)).
  - Targets are sorted by their (colsA, colsB) pair (snake order) into
    blocks of 256; per-block plane counts are static (max over the 8 cores).
  - Per (block, pair) plane group: DVE strided reduce over the planes
    collapses the neighbor slots to a bf16 [128, 256] sum.
  - PE (all bf16, 1 cycle/row): per block, psum[64,256] accumulates
    self term (lhsT=W0^T, rhs = deg-prescaled x_self) + [W1^T;W1^T] @ sumA
    + [W1^T;W1^T] @ sumB.  The deg prescale makes a single 1/deg
    post-multiply correct for the whole psum: (deg*W0x + sum W1x_nbr)/deg.
  - ScalarE spills psum to partitions 64..127 of an output staging tile;
    DVE multiplies 4 blocks at once by 1/deg (bf16, replicated on
    partitions 64..127 of the combo tile -- equal SB base partitions);
    ScalarE adds the bias; DMA writes [64, 1024] f32 per 4 blocks.
    Host un-permutes output columns.
"""

import numpy as np
import ml_dtypes

import concourse.bacc as bacc
import concourse.mybir as mybir
from concourse.tile import TileContext
from concourse.bass_utils import run_bass_kernel_spmd

B, C, V, D, O = 8, 64, 25000, 12, 64
VP = 25088            # padded target count (196 blocks of 128)
BLK = 128
NB = VP // BLK        # 196
NE = 6272             # table entries per quarter (zero column at NE-1)
ZID = NE - 1
CHUNK_COLS_MIN = 6272  # ap_gather cost floor: max(num_elems, num_idxs)
CHUNK_COLS_CAP = 6656  # SBUF bound for the gather tile (26 KiB/partition f32)
MAX_BLOCK_GAP = 8      # keeps the A/B block frontiers close
OUT_GRP = 8            # blocks per output staging flush
DIRECT_N = 3           # block-completing groups this small skip the DVE
                       # reduce: PE accumulates their f32 planes directly

f32 = mybir.dt.float32
bf16 = mybir.dt.bfloat16
i16 = mybir.dt.int16


# ---------------------------------------------------------------- host prep

def _edge_arrays(nbr, dg):
    """Sorted-by-source edge list: for vertex v, targets tgt_s[starts[v]:starts[v+1]]."""
    mask = np.arange(D)[None, :] < dg[:, None]
    tgt = np.repeat(np.arange(V), D)[mask.ravel()]
    src = nbr.ravel()[mask.ravel()]
    order = np.argsort(src, kind="stable")
    src_s, tgt_s = src[order], tgt[order]
    starts = np.searchsorted(src_s, np.arange(V + 1))
    return tgt_s.astype(np.int64), starts.astype(np.int64)


def _ranges(starts, lens):
    """Concatenated arange(starts[i], starts[i]+lens[i])."""
    tot = int(lens.sum())
    if tot == 0:
        return np.zeros(0, np.int64)
    csum = np.zeros(len(lens), np.int64)
    csum[1:] = lens.cumsum()[:-1]
    return np.repeat(starts, lens) + np.arange(tot) - np.repeat(csum, lens)


def balance_quarters(nbr, dg, seed, sweeps=2):
    """Assign each vertex to a quarter (0..3; pairs (0,1) and (2,3)) so that
    each target list splits evenly across its pair's halves.  Exact
    sequential greedy on the incremental column cost (1 + sweeps passes)."""
    tgt_s, starts = _edge_arrays(nbr, dg)
    cnt = np.zeros((V, 4), np.int32)
    q = np.full(V, -1, np.int8)
    caps = np.zeros(4, np.int64)
    CAPQ = NE - 1
    rng = np.random.default_rng(seed)
    vorder = rng.permutation(V)
    for it in range(1 + sweeps):
        for v in vorder:
            lists = tgt_s[starts[v] : starts[v + 1]]
            c = cnt[lists].astype(np.int32)
            if q[v] >= 0:
                # remove ALL of v's own occurrences per duplicate target row
                uniq, ucnt = np.unique(lists, return_counts=True)
                c[:, q[v]] -= ucnt[np.searchsorted(uniq, lists)].astype(np.int32)
            best, bestkey = -1, None
            for k in range(4):
                if caps[k] >= CAPQ + (1 if q[v] == k else 0):
                    continue
                pk = k ^ 1
                m = int((c[:, k] >= c[:, pk]).sum())
                key = (m, int(caps[k]))
                if bestkey is None or key < bestkey:
                    best, bestkey = k, key
            if best == q[v]:
                continue
            if q[v] >= 0:
                np.subtract.at(cnt, (lists, q[v]), 1)
                caps[q[v]] -= 1
            q[v] = best
            caps[best] += 1
            np.add.at(cnt, (lists, best), 1)
    assert (np.bincount(q.astype(np.int64), minlength=4) <= CAPQ).all()
    colsA = np.maximum(cnt[:, 0], cnt[:, 1])
    colsB = np.maximum(cnt[:, 2], cnt[:, 3])
    return q, colsA, colsB


def _snake_order(colsA, colsB):
    """Sort targets (padded to VP) so blocks are homogeneous in (colsA, colsB)."""
    cA = np.zeros(VP, np.int64)
    cA[:V] = colsA
    cB = np.zeros(VP, np.int64)
    cB[:V] = colsB
    snake = cB.copy()
    odd = cA % 2 == 1
    snake[odd] = 63 - snake[odd]
    key = cA * 64 + snake
    order = np.argsort(key, kind="stable")
    return order, cA, cB


def make_schedule(PA, PB):
    """Static chunk schedule. Chunks alternate between the pair-A and pair-B
    plane streams, contain only WHOLE per-block plane groups (a block's pair
    reduce is one strided view over one chunk tile), and are packed to at
    least CHUNK_COLS_MIN columns (the ap_gather cost floor).

    Returns chunks = [(pair, [(block, nplanes), ...])] in issue order."""
    groupsA = [(b, int(PA[b])) for b in range(NB) if PA[b] > 0]
    groupsB = [(b, int(PB[b])) for b in range(NB) if PB[b] > 0]
    chunks = []
    while groupsA or groupsB:
        frontA = groupsA[0][0] if groupsA else NB
        frontB = groupsB[0][0] if groupsB else NB
        pick_a = frontA <= frontB
        if len(chunks) < 1:
            # pair-B table half loads while the first A-chunk gathers
            pick_a = True
        if not groupsA:
            pick_a = False
        if not groupsB:
            pick_a = True
        gs = groupsA if pick_a else groupsB
        # even out chunk sizes so stream tails don't pay the cost floor
        rem = sum(n for _, n in gs) * BLK
        nch = max(1, -(-rem // CHUNK_COLS_CAP))
        target = -(-rem // nch)
        take, cols = [], 0
        while gs:
            gcols = gs[0][1] * BLK
            if cols + gcols > CHUNK_COLS_CAP or (cols >= target and take):
                break
            take.append(gs.pop(0))
            cols += gcols
        chunks.append(("A" if pick_a else "B", take))
    return chunks


def _chunk_cols(chunk):
    return sum(n for _, n in chunk[1]) * BLK


# ---------------------------------------------------------------- device

def build_nc(PA, PB):
    chunks = make_schedule(PA, PB)
    tot_cols = sum(_chunk_cols(c) for c in chunks)

    nc = bacc.Bacc()
    tbl_d = nc.declare_dram_parameter("tbl", [128, 2 * NE], f32, isOutput=False)
    combo_d = nc.declare_dram_parameter("combo", [128, VP], bf16, isOutput=False)
    idx_d = nc.declare_dram_parameter("idxs", [128, tot_cols // 16], i16, isOutput=False)
    w1cat_d = nc.declare_dram_parameter("w1cat", [128, O], bf16, isOutput=False)
    w1f_d = nc.declare_dram_parameter("w1f", [128, O], f32, isOutput=False)
    w0t_d = nc.declare_dram_parameter("w0t", [C, O], bf16, isOutput=False)
    bias_d = nc.declare_dram_parameter("bias", [O, 1], f32, isOutput=False)
    out_d = nc.declare_dram_parameter("out", [O, VP], f32, isOutput=True)

    ngroups = [int(PA[b] > 0) + int(PB[b] > 0) for b in range(NB)]

    with TileContext(nc) as tc:
        with (
            tc.tile_pool(name="const", bufs=1) as cpool,
            tc.tile_pool(name="idxp", bufs=3) as idxpool,
            tc.tile_pool(name="gp", bufs=3) as gpool,
            tc.tile_pool(name="sump", bufs=48) as sumpool,
            tc.tile_pool(name="outp", bufs=3) as outpool,
            tc.tile_pool(name="ps", bufs=4, space="PSUM") as pspool,
        ):
            # DMAs serialize on the engine pool in program order, so only the
            # pair-A table half + small weights go before the first chunk's
            # idx load; the B half and combo segments interleave with chunks
            tbl = cpool.tile([128, 2 * NE], f32)
            nc.sync.dma_start(out=tbl[:, 0:NE], in_=tbl_d[:, 0:NE])
            w1cat = cpool.tile([128, O], bf16)
            nc.sync.dma_start(out=w1cat[:, :], in_=w1cat_d[:, :])
            w1f = cpool.tile([128, O], f32)
            nc.sync.dma_start(out=w1f[:, :], in_=w1f_d[:, :])
            w0t = cpool.tile([C, O], bf16)
            nc.sync.dma_start(out=w0t[:, :], in_=w0t_d[:, :])
            # bias on partitions 64..127 to match the staging tile's base
            bb = cpool.tile([2 * O, 1], f32)
            nc.sync.dma_start(out=bb[O : 2 * O, :], in_=bias_d[:, :])
            combo = cpool.tile([128, VP], bf16)
            SEG = VP // 8
            combo_loaded = [0]  # columns of combo DMA'd so far

            def ensure_combo(cols_needed):
                while combo_loaded[0] < min(cols_needed, VP):
                    s0 = combo_loaded[0]
                    s1 = min(s0 + SEG, VP)
                    nc.sync.dma_start(out=combo[:, s0:s1], in_=combo_d[:, s0:s1])
                    combo_loaded[0] = s1

            def deferred_loads(k, groups):
                if k == 0:
                    nc.sync.dma_start(
                        out=tbl[:, NE : 2 * NE], in_=tbl_d[:, NE : 2 * NE]
                    )
                # any group completing in this chunk may emit its block:
                # cover every block this chunk touches (+1 segment lookahead)
                max_b = max(b_ for b_, _ in groups)
                ensure_combo((max_b + 1) * BLK + SEG)

            sums = {}           # block -> [bf16 sum tiles]
            spilled = [False] * NB
            out_tiles = {}      # OUT_GRP-block group -> [tile, spill count]
            n1_flip = [0]
            idx_off = [0]

            def emit_block(b, direct=None):
                """All sums ready: self + neighbor matmuls back-to-back, then
                spill.  psum lives only for this instruction run.  `direct` =
                (g_tile, c0, n): the completing group's f32 planes feed PE
                straight from the gather tile (no DVE reduce)."""
                ss = sums.pop(b, [])
                nmm = len(ss) + (direct[2] if direct else 0)
                ps = pspool.tile([O, BLK], f32, tag="ps", name="ps")
                nc.tensor.matmul(
                    ps[:, :],
                    lhsT=w0t[:, :],
                    rhs=combo[0:C, b * BLK : (b + 1) * BLK],
                    start=True,
                    stop=(nmm == 0),
                )
                for j, sm in enumerate(ss):
                    nc.tensor.matmul(
                        ps[:, :], lhsT=w1cat[:, :], rhs=sm[:, :],
                        start=False, stop=(j == nmm - 1),
                    )
                if direct is not None:
                    dg, dc0, dn = direct
                    for p in range(dn):
                        nc.tensor.matmul(
                            ps[:, :],
                            lhsT=w1f[:, :],
                            rhs=dg[:, dc0 + p * BLK : dc0 + (p + 1) * BLK],
                            start=False,
                            stop=(len(ss) + p == nmm - 1),
                        )
                # spill to staging rows 64..127; flush when the group is full
                grp = b // OUT_GRP
                gsize = min(OUT_GRP, NB - grp * OUT_GRP)
                if grp not in out_tiles:
                    t = outpool.tile(
                        [128, OUT_GRP * BLK], f32, tag="outst", name="outst"
                    )
                    out_tiles[grp] = [t, 0]
                ent = out_tiles[grp]
                slot = b % OUT_GRP
                nc.scalar.copy(
                    ent[0][O : 2 * O, slot * BLK : (slot + 1) * BLK], ps[:, :]
                )
                spilled[b] = True
                ent[1] += 1
                if ent[1] == gsize:
                    base = grp * OUT_GRP
                    n = gsize * BLK
                    hi = ent[0][O : 2 * O, 0:n]
                    nc.vector.tensor_mul(
                        hi, hi, combo[C : 2 * C, base * BLK : base * BLK + n]
                    )
                    nc.scalar.add(hi, hi, add=bb[O : 2 * O, 0:1])
                    nc.sync.dma_start(
                        out=out_d[:, base * BLK : base * BLK + n], in_=hi
                    )
                    del out_tiles[grp]

            idx_tiles = {}

            def load_idx(k):
                if k >= len(chunks):
                    return
                icols = _chunk_cols(chunks[k]) // 16
                t = idxpool.tile([128, icols], i16, tag="idxt", name="idxt")
                nc.sync.dma_start(
                    out=t[:, :], in_=idx_d[:, idx_off[0] : idx_off[0] + icols]
                )
                idx_off[0] += icols
                idx_tiles[k] = t

            load_idx(0)
            for ck, (pair, groups) in enumerate(chunks):
                ncols = sum(n for _, n in groups) * BLK
                idxt = idx_tiles.pop(ck)
                g = gpool.tile([128, ncols], f32, tag="g", name="g")
                te = tbl[:, 0:NE] if pair == "A" else tbl[:, NE : 2 * NE]
                nc.gpsimd.ap_gather(
                    g[:, :], te, idxt[:, :],
                    channels=128, num_elems=NE, d=1, num_idxs=ncols,
                )
                # next chunk's idx DMA queues before this chunk's bulk loads
                load_idx(ck + 1)
                deferred_loads(ck, groups)
                c0 = 0
                for b, n in groups:
                    completes = len(sums.get(b, [])) + 1 == ngroups[b]
                    if completes and n <= DIRECT_N:
                        emit_block(b, direct=(g, c0, n))
                        c0 += n * BLK
                        continue
                    sm = sumpool.tile([128, BLK], bf16, tag="sm", name="sm")
                    with nc.allow_low_precision(reason="bf16 slot sums; 2e-2 budget"):
                        if n == 1:
                            # ScalarE handles these: keeps DVE free to drain
                            # reduces so gather tiles recycle sooner
                            nc.scalar.copy(sm[:, :], g[:, c0 : c0 + BLK])
                        else:
                            nc.vector.reduce_sum(
                                out=sm[:, :],
                                in_=g[:, c0 : c0 + n * BLK].rearrange(
                                    "p (n c) -> p c n", c=BLK
                                ),
                                axis=mybir.AxisListType.X,
                            )
                    sums.setdefault(b, []).append(sm)
                    if len(sums[b]) == ngroups[b]:
                        emit_block(b)
                    c0 += n * BLK
            # blocks untouched by either stream (pads / self-only)
            for b in range(NB):
                if not spilled[b]:
                    emit_block(b)
    nc.finalize()
    return nc


# ---------------------------------------------------------------- per-call

def _profile_and_maps(x, nbr_idx, deg, W, b):
    """Balance + sort + shared plane profile + per-core input maps."""
    per_core = []
    for bi in range(B):
        q, colsA, colsB = balance_quarters(nbr_idx[bi], deg[bi], seed=1234 + bi)
        order, cA, cB = _snake_order(colsA, colsB)
        per_core.append((q, order, cA, cB))

    planesA_pc = np.zeros((B, NB), np.int64)
    planesB_pc = np.zeros((B, NB), np.int64)
    for bi in range(B):
        _, order, cA, cB = per_core[bi]
        planesA_pc[bi] = cA[order].reshape(NB, BLK).max(1)
        planesB_pc[bi] = cB[order].reshape(NB, BLK).max(1)
    PA = planesA_pc.max(0)
    PB = planesB_pc.max(0)
    chunks = make_schedule(PA, PB)

    w1 = np.ascontiguousarray(W[:, :, 1].T, np.float32)      # [C, O]
    w0 = np.ascontiguousarray(W[:, :, 0].T, np.float32)
    w1cat = np.concatenate([w1, w1], axis=0)                  # [128, O]
    bvec = np.ascontiguousarray(b.reshape(O, 1), np.float32)

    in_maps = []
    for bi in range(B):
        q, order, cA, cB = per_core[bi]
        xb = x[bi]                                            # [C, V]
        qlocal = np.zeros(V, np.int64)
        tblf = np.zeros((128, 2 * NE), np.float32)
        for k in range(4):
            verts = np.where(q == k)[0]
            qlocal[verts] = np.arange(len(verts))
            rows = slice(0, C) if k % 2 == 0 else slice(C, 2 * C)
            cols0 = 0 if k < 2 else NE
            tblf[rows, cols0 : cols0 + len(verts)] = xb[:, verts]
        # per-target per-quarter ranked neighbor slots [4, VP, maxP]
        maxP = int(max(PA.max(), PB.max(), 1))
        mats = np.full((4, VP, maxP), ZID, np.int16)
        mask = np.arange(D)[None, :] < deg[bi][:, None]
        u_e = np.repeat(np.arange(V), D)[mask.ravel()]
        n_e = nbr_idx[bi].ravel()[mask.ravel()]
        k_e = q[n_e].astype(np.int64)
        key = u_e.astype(np.int64) * 4 + k_e
        o2 = np.argsort(key, kind="stable")
        u_s, n_s, k_s = u_e[o2], n_e[o2], k_e[o2]
        key_s = key[o2]
        first = np.searchsorted(key_s, key_s, side="left")
        rank = np.arange(len(key_s)) - first
        mats[k_s, u_s, rank] = qlocal[n_s].astype(np.int16)
        assert mats.max() <= ZID and mats.min() >= 0

        tgt_mat = order.reshape(NB, BLK)
        wrapped_parts = []
        for pair, groups in chunks:
            bs = np.concatenate([[b_] * n for b_, n in groups]).astype(np.int64)
            ps = np.concatenate([np.arange(n) for _, n in groups]).astype(np.int64)
            mlow, mhigh = (mats[0], mats[1]) if pair == "A" else (mats[2], mats[3])
            tgts = tgt_mat[bs]                                 # [nplanes, BLK]
            low = mlow[tgts, ps[:, None]].reshape(-1)
            high = mhigh[tgts, ps[:, None]].reshape(-1)
            ncols = len(bs) * BLK
            wl = low.reshape(ncols // 16, 16).T
            wh = high.reshape(ncols // 16, 16).T
            wrap = np.concatenate([np.tile(wl, (4, 1)), np.tile(wh, (4, 1))], axis=0)
            wrapped_parts.append(wrap.astype(np.int16))
        idxs = np.ascontiguousarray(np.concatenate(wrapped_parts, axis=1))

        degf = np.maximum(deg[bi], 1).astype(np.float32)
        invd_vp = np.ones(VP, np.float32)
        invd_vp[:V] = 1.0 / degf
        deg_vp = np.ones(VP, np.float32)
        deg_vp[:V] = degf
        xs_vp = np.zeros((C, VP), np.float32)
        valid = order < V
        xs_vp[:, valid] = xb[:, order[valid]]
        # self term pre-scaled by deg: psum = deg*W0*x + sum W1*x_nbr, then *1/deg
        xs_vp *= deg_vp[order][None, :]
        combo = np.concatenate(
            [xs_vp, np.tile(invd_vp[order][None, :], (C, 1))], axis=0
        ).astype(ml_dtypes.bfloat16)

        in_maps.append(
            {
                "tbl": tblf,
                "combo": combo,
                "idxs": idxs,
                "w1cat": w1cat.astype(ml_dtypes.bfloat16),
                "w1f": w1cat,
                "w0t": w0.astype(ml_dtypes.bfloat16),
                "bias": bvec,
            }
        )
    orders = [pc[1] for pc in per_core]
    return (tuple(PA.tolist()), tuple(PB.tolist())), in_maps, orders


_CACHE = {}
TRACE = False
LAST_RESULT = None


def _get_nc(PA, PB):
    key = (PA, PB)
    if key not in _CACHE:
        _CACHE[key] = build_nc(np.array(PA), np.array(PB))
    return _CACHE[key]


def kernel(x, nbr_idx, deg, W, b):
    global LAST_RESULT
    x = np.asarray(x, np.float32)
    nbr_idx = np.asarray(nbr_idx, np.int32)
    deg = np.asarray(deg, np.int32)
    W = np.asarray(W, np.float32)
    b = np.asarray(b, np.float32)
    (PA, PB), in_maps, orders = _profile_and_maps(x, nbr_idx, deg, W, b)
    nc = _get_nc(PA, PB)
    try:
        res = run_bass_kernel_spmd(nc, in_maps, list(range(len(in_maps))), trace=TRACE)
    except ModuleNotFoundError:
        res = run_bass_kernel_spmd(nc, in_maps, list(range(len(in_maps))), trace=False)
    LAST_RESULT = res
    outs = []
    for bi, r in enumerate(res.results):
        order = orders[bi]
        valid = order < V
        ob = np.empty((O, V), np.float32)
        ob[:, order[valid]] = r["out"][:, valid]
        outs.append(ob)
    out = np.stack(outs, axis=0)
    return out[..., None].astype(np.float32)


# revision 61
# speedup vs baseline: 1.0617x; 1.0617x over previous
"""MeshConvPoint Bass/Trainium2 kernel — on-chip GPSIMD gather architecture.

Problem (per mesh b of B=8, one NeuronCore each):
    nbr_mean[c,v] = (1/deg[v]) * sum_{d<deg[v]} x[c, nbr_idx[v,d]]
    out[o,v]     = sum_c W0[o,c]*x[c,v] + W1[o,c]*nbr_mean[c,v] + b[o]

Strategy (replaces the SWDGE dma_gather baseline, which was DMA-descriptor
bound at ~233us: 256B/descriptor * 22.76ns / 16 engines * ~164k edges):

  - x lives in SBUF channel-major as a quartered table [128, 2*NE] f32:
    pair A = cols 0:NE (partitions 0..63 = quarter Q0, 64..127 = Q1),
    pair B = cols NE:2NE (Q2 / Q3).  One GPSIMD ap_gather column fetches
    TWO neighbors of the same target vertex (one per table half) at
    ~1.39ns/column -- data never leaves SBUF.
  - A host-side greedy balance assigns each vertex to a quarter so that each
    target's neighbor list splits evenly across the halves of its pair
    (columns per target ~= ceil(deg/2)).
  - Targets are sorted by their (colsA, colsB) pair (snake order) into
    blocks of 256; per-block plane counts are static (max over the 8 cores).
  - Per (block, pair) plane group: DVE strided reduce over the planes
    collapses the neighbor slots to a bf16 [128, 256] sum.
  - PE (all bf16, 1 cycle/row): per block, psum[64,256] accumulates
    self term (lhsT=W0^T, rhs = deg-prescaled x_self) + [W1^T;W1^T] @ sumA
    + [W1^T;W1^T] @ sumB.  The deg prescale makes a single 1/deg
    post-multiply correct for the whole psum: (deg*W0x + sum W1x_nbr)/deg.
  - ScalarE spills psum to partitions 64..127 of an output staging tile;
    DVE multiplies 4 blocks at once by 1/deg (bf16, replicated on
    partitions 64..127 of the combo tile -- equal SB base partitions);
    ScalarE adds the bias; DMA writes [64, 1024] f32 per 4 blocks.
    Host un-permutes output columns.
"""

import numpy as np
import ml_dtypes

import concourse.bacc as bacc
import concourse.mybir as mybir
from concourse.tile import TileContext
from concourse.bass_utils import run_bass_kernel_spmd

B, C, V, D, O = 8, 64, 25000, 12, 64
VP = 25088            # padded target count (98 blocks of 256)
BLK = 256
NB = VP // BLK        # 98
NE = 6272             # table entries per quarter (zero column at NE-1)
ZID = NE - 1
CHUNK_COLS_MIN = 6272  # ap_gather cost floor: max(num_elems, num_idxs)
CHUNK_COLS_CAP = 6656  # SBUF bound for the gather tile (26 KiB/partition f32)
MAX_BLOCK_GAP = 8      # keeps the A/B block frontiers close
OUT_GRP = 4            # blocks per output staging flush
DIRECT_N = 3           # block-completing groups this small skip the DVE
                       # reduce: PE accumulates their f32 planes directly

f32 = mybir.dt.float32
bf16 = mybir.dt.bfloat16
i16 = mybir.dt.int16


# ---------------------------------------------------------------- host prep

def _edge_arrays(nbr, dg):
    """Sorted-by-source edge list: for vertex v, targets tgt_s[starts[v]:starts[v+1]]."""
    mask = np.arange(D)[None, :] < dg[:, None]
    tgt = np.repeat(np.arange(V), D)[mask.ravel()]
    src = nbr.ravel()[mask.ravel()]
    order = np.argsort(src, kind="stable")
    src_s, tgt_s = src[order], tgt[order]
    starts = np.searchsorted(src_s, np.arange(V + 1))
    return tgt_s.astype(np.int64), starts.astype(np.int64)


def _ranges(starts, lens):
    """Concatenated arange(starts[i], starts[i]+lens[i])."""
    tot = int(lens.sum())
    if tot == 0:
        return np.zeros(0, np.int64)
    csum = np.zeros(len(lens), np.int64)
    csum[1:] = lens.cumsum()[:-1]
    return np.repeat(starts, lens) + np.arange(tot) - np.repeat(csum, lens)


def balance_quarters(nbr, dg, seed, sweeps=2):
    """Assign each vertex to a quarter (0..3; pairs (0,1) and (2,3)) so that
    each target list splits evenly across its pair's halves.  Exact
    sequential greedy on the incremental column cost (1 + sweeps passes)."""
    tgt_s, starts = _edge_arrays(nbr, dg)
    cnt = np.zeros((V, 4), np.int32)
    q = np.full(V, -1, np.int8)
    caps = np.zeros(4, np.int64)
    CAPQ = NE - 1
    rng = np.random.default_rng(seed)
    vorder = rng.permutation(V)
    for it in range(1 + sweeps):
        for v in vorder:
            lists = tgt_s[starts[v] : starts[v + 1]]
            c = cnt[lists].astype(np.int32)
            if q[v] >= 0:
                # remove ALL of v's own occurrences per duplicate target row
                uniq, ucnt = np.unique(lists, return_counts=True)
                c[:, q[v]] -= ucnt[np.searchsorted(uniq, lists)].astype(np.int32)
            best, bestkey = -1, None
            for k in range(4):
                if caps[k] >= CAPQ + (1 if q[v] == k else 0):
                    continue
                pk = k ^ 1
                m = int((c[:, k] >= c[:, pk]).sum())
                key = (m, int(caps[k]))
                if bestkey is None or key < bestkey:
                    best, bestkey = k, key
            if best == q[v]:
                continue
            if q[v] >= 0:
                np.subtract.at(cnt, (lists, q[v]), 1)
                caps[q[v]] -= 1
            q[v] = best
            caps[best] += 1
            np.add.at(cnt, (lists, best), 1)
    assert (np.bincount(q.astype(np.int64), minlength=4) <= CAPQ).all()
    colsA = np.maximum(cnt[:, 0], cnt[:, 1])
    colsB = np.maximum(cnt[:, 2], cnt[:, 3])
    return q, colsA, colsB


def _snake_order(colsA, colsB):
    """Sort targets (padded to VP) so blocks are homogeneous in (colsA, colsB)."""
    cA = np.zeros(VP, np.int64)
    cA[:V] = colsA
    cB = np.zeros(VP, np.int64)
    cB[:V] = colsB
    snake = cB.copy()
    odd = cA % 2 == 1
    snake[odd] = 63 - snake[odd]
    key = cA * 64 + snake
    order = np.argsort(key, kind="stable")
    return order, cA, cB


def make_schedule(PA, PB):
    """Static chunk schedule. Chunks alternate between the pair-A and pair-B
    plane streams, contain only WHOLE per-block plane groups (a block's pair
    reduce is one strided view over one chunk tile), and are packed to at
    least CHUNK_COLS_MIN columns (the ap_gather cost floor).

    Returns chunks = [(pair, [(block, nplanes), ...])] in issue order."""
    groupsA = [(b, int(PA[b])) for b in range(NB) if PA[b] > 0]
    groupsB = [(b, int(PB[b])) for b in range(NB) if PB[b] > 0]
    chunks = []
    while groupsA or groupsB:
        frontA = groupsA[0][0] if groupsA else NB
        frontB = groupsB[0][0] if groupsB else NB
        pick_a = frontA <= frontB
        if len(chunks) < 1:
            # pair-B table half loads while the first A-chunk gathers
            pick_a = True
        if not groupsA:
            pick_a = False
        if not groupsB:
            pick_a = True
        gs = groupsA if pick_a else groupsB
        # even out chunk sizes so stream tails don't pay the cost floor
        rem = sum(n for _, n in gs) * BLK
        nch = max(1, -(-rem // CHUNK_COLS_CAP))
        target = -(-rem // nch)
        take, cols = [], 0
        while gs:
            gcols = gs[0][1] * BLK
            if cols + gcols > CHUNK_COLS_CAP or (cols >= target and take):
                break
            take.append(gs.pop(0))
            cols += gcols
        chunks.append(("A" if pick_a else "B", take))
    return chunks


def _chunk_cols(chunk):
    return sum(n for _, n in chunk[1]) * BLK


# ---------------------------------------------------------------- device

def build_nc(PA, PB):
    chunks = make_schedule(PA, PB)
    tot_cols = sum(_chunk_cols(c) for c in chunks)

    nc = bacc.Bacc()
    tbl_d = nc.declare_dram_parameter("tbl", [128, 2 * NE], f32, isOutput=False)
    combo_d = nc.declare_dram_parameter("combo", [128, VP], bf16, isOutput=False)
    idx_d = nc.declare_dram_parameter("idxs", [128, tot_cols // 16], i16, isOutput=False)
    w1cat_d = nc.declare_dram_parameter("w1cat", [128, O], bf16, isOutput=False)
    w1f_d = nc.declare_dram_parameter("w1f", [128, O], f32, isOutput=False)
    w0t_d = nc.declare_dram_parameter("w0t", [C, O], bf16, isOutput=False)
    bias_d = nc.declare_dram_parameter("bias", [O, 1], f32, isOutput=False)
    out_d = nc.declare_dram_parameter("out", [O, VP], f32, isOutput=True)

    ngroups = [int(PA[b] > 0) + int(PB[b] > 0) for b in range(NB)]

    with TileContext(nc) as tc:
        with (
            tc.tile_pool(name="const", bufs=1) as cpool,
            tc.tile_pool(name="idxp", bufs=3) as idxpool,
            tc.tile_pool(name="gp", bufs=3) as gpool,
            tc.tile_pool(name="sump", bufs=26) as sumpool,
            tc.tile_pool(name="outp", bufs=3) as outpool,
            tc.tile_pool(name="ps", bufs=4, space="PSUM") as pspool,
        ):
            # DMAs serialize on the engine pool in program order, so only the
            # pair-A table half + small weights go before the first chunk's
            # idx load; the B half and combo segments interleave with chunks
            tbl = cpool.tile([128, 2 * NE], f32)
            nc.sync.dma_start(out=tbl[:, 0:NE], in_=tbl_d[:, 0:NE])
            w1cat = cpool.tile([128, O], bf16)
            nc.sync.dma_start(out=w1cat[:, :], in_=w1cat_d[:, :])
            w1f = cpool.tile([128, O], f32)
            nc.sync.dma_start(out=w1f[:, :], in_=w1f_d[:, :])
            w0t = cpool.tile([C, O], bf16)
            nc.sync.dma_start(out=w0t[:, :], in_=w0t_d[:, :])
            # bias on partitions 64..127 to match the staging tile's base
            bb = cpool.tile([2 * O, 1], f32)
            nc.sync.dma_start(out=bb[O : 2 * O, :], in_=bias_d[:, :])
            combo = cpool.tile([128, VP], bf16)
            SEG = VP // 8
            combo_loaded = [0]  # columns of combo DMA'd so far

            def ensure_combo(cols_needed):
                while combo_loaded[0] < min(cols_needed, VP):
                    s0 = combo_loaded[0]
                    s1 = min(s0 + SEG, VP)
                    nc.sync.dma_start(out=combo[:, s0:s1], in_=combo_d[:, s0:s1])
                    combo_loaded[0] = s1

            def deferred_loads(k, groups):
                if k == 0:
                    nc.sync.dma_start(
                        out=tbl[:, NE : 2 * NE], in_=tbl_d[:, NE : 2 * NE]
                    )
                # any group completing in this chunk may emit its block:
                # cover every block this chunk touches (+1 segment lookahead)
                max_b = max(b_ for b_, _ in groups)
                ensure_combo((max_b + 1) * BLK + SEG)

            sums = {}           # block -> [bf16 sum tiles]
            spilled = [False] * NB
            out_tiles = {}      # OUT_GRP-block group -> [tile, spill count]
            n1_flip = [0]
            idx_off = [0]

            def emit_block(b, direct=None):
                """All sums ready: self + neighbor matmuls back-to-back, then
                spill.  psum lives only for this instruction run.  `direct` =
                (g_tile, c0, n): the completing group's f32 planes feed PE
                straight from the gather tile (no DVE reduce)."""
                ss = sums.pop(b, [])
                nmm = len(ss) + (direct[2] if direct else 0)
                ps = pspool.tile([O, BLK], f32, tag="ps", name="ps")
                nc.tensor.matmul(
                    ps[:, :],
                    lhsT=w0t[:, :],
                    rhs=combo[0:C, b * BLK : (b + 1) * BLK],
                    start=True,
                    stop=(nmm == 0),
                )
                for j, sm in enumerate(ss):
                    nc.tensor.matmul(
                        ps[:, :], lhsT=w1cat[:, :], rhs=sm[:, :],
                        start=False, stop=(j == nmm - 1),
                    )
                if direct is not None:
                    dg, dc0, dn = direct
                    for p in range(dn):
                        nc.tensor.matmul(
                            ps[:, :],
                            lhsT=w1f[:, :],
                            rhs=dg[:, dc0 + p * BLK : dc0 + (p + 1) * BLK],
                            start=False,
                            stop=(len(ss) + p == nmm - 1),
                        )
                # spill to staging rows 64..127; flush when the group is full
                grp = b // OUT_GRP
                gsize = min(OUT_GRP, NB - grp * OUT_GRP)
                if grp not in out_tiles:
                    t = outpool.tile(
                        [128, OUT_GRP * BLK], f32, tag="outst", name="outst"
                    )
                    out_tiles[grp] = [t, 0]
                ent = out_tiles[grp]
                slot = b % OUT_GRP
                nc.scalar.copy(
                    ent[0][O : 2 * O, slot * BLK : (slot + 1) * BLK], ps[:, :]
                )
                spilled[b] = True
                ent[1] += 1
                if ent[1] == gsize:
                    base = grp * OUT_GRP
                    n = gsize * BLK
                    hi = ent[0][O : 2 * O, 0:n]
                    nc.vector.tensor_mul(
                        hi, hi, combo[C : 2 * C, base * BLK : base * BLK + n]
                    )
                    nc.scalar.add(hi, hi, add=bb[O : 2 * O, 0:1])
                    nc.sync.dma_start(
                        out=out_d[:, base * BLK : base * BLK + n], in_=hi
                    )
                    del out_tiles[grp]

            idx_tiles = {}

            def load_idx(k):
                if k >= len(chunks):
                    return
                icols = _chunk_cols(chunks[k]) // 16
                t = idxpool.tile([128, icols], i16, tag="idxt", name="idxt")
                nc.sync.dma_start(
                    out=t[:, :], in_=idx_d[:, idx_off[0] : idx_off[0] + icols]
                )
                idx_off[0] += icols
                idx_tiles[k] = t

            load_idx(0)
            for ck, (pair, groups) in enumerate(chunks):
                ncols = sum(n for _, n in groups) * BLK
                idxt = idx_tiles.pop(ck)
                g = gpool.tile([128, ncols], f32, tag="g", name="g")
                te = tbl[:, 0:NE] if pair == "A" else tbl[:, NE : 2 * NE]
                nc.gpsimd.ap_gather(
                    g[:, :], te, idxt[:, :],
                    channels=128, num_elems=NE, d=1, num_idxs=ncols,
                )
                # next chunk's idx DMA queues before this chunk's bulk loads
                load_idx(ck + 1)
                deferred_loads(ck, groups)
                c0 = 0
                for b, n in groups:
                    completes = len(sums.get(b, [])) + 1 == ngroups[b]
                    if completes and n <= DIRECT_N:
                        emit_block(b, direct=(g, c0, n))
                        c0 += n * BLK
                        continue
                    sm = sumpool.tile([128, BLK], bf16, tag="sm", name="sm")
                    with nc.allow_low_precision(reason="bf16 slot sums; 2e-2 budget"):
                        if n == 1:
                            # ScalarE handles these: keeps DVE free to drain
                            # reduces so gather tiles recycle sooner
                            nc.scalar.copy(sm[:, :], g[:, c0 : c0 + BLK])
                        else:
                            nc.vector.reduce_sum(
                                out=sm[:, :],
                                in_=g[:, c0 : c0 + n * BLK].rearrange(
                                    "p (n c) -> p c n", c=BLK
                                ),
                                axis=mybir.AxisListType.X,
                            )
                    sums.setdefault(b, []).append(sm)
                    if len(sums[b]) == ngroups[b]:
                        emit_block(b)
                    c0 += n * BLK
            # blocks untouched by either stream (pads / self-only)
            for b in range(NB):
                if not spilled[b]:
                    emit_block(b)
    nc.finalize()
    return nc


# ---------------------------------------------------------------- per-call

def _profile_and_maps(x, nbr_idx, deg, W, b):
    """Balance + sort + shared plane profile + per-core input maps."""
    per_core = []
    for bi in range(B):
        q, colsA, colsB = balance_quarters(nbr_idx[bi], deg[bi], seed=1234 + bi)
        order, cA, cB = _snake_order(colsA, colsB)
        per_core.append((q, order, cA, cB))

    planesA_pc = np.zeros((B, NB), np.int64)
    planesB_pc = np.zeros((B, NB), np.int64)
    for bi in range(B):
        _, order, cA, cB = per_core[bi]
        planesA_pc[bi] = cA[order].reshape(NB, BLK).max(1)
        planesB_pc[bi] = cB[order].reshape(NB, BLK).max(1)
    PA = planesA_pc.max(0)
    PB = planesB_pc.max(0)
    chunks = make_schedule(PA, PB)

    w1 = np.ascontiguousarray(W[:, :, 1].T, np.float32)      # [C, O]
    w0 = np.ascontiguousarray(W[:, :, 0].T, np.float32)
    w1cat = np.concatenate([w1, w1], axis=0)                  # [128, O]
    bvec = np.ascontiguousarray(b.reshape(O, 1), np.float32)

    in_maps = []
    for bi in range(B):
        q, order, cA, cB = per_core[bi]
        xb = x[bi]                                            # [C, V]
        qlocal = np.zeros(V, np.int64)
        tblf = np.zeros((128, 2 * NE), np.float32)
        for k in range(4):
            verts = np.where(q == k)[0]
            qlocal[verts] = np.arange(len(verts))
            rows = slice(0, C) if k % 2 == 0 else slice(C, 2 * C)
            cols0 = 0 if k < 2 else NE
            tblf[rows, cols0 : cols0 + len(verts)] = xb[:, verts]
        # per-target per-quarter ranked neighbor slots [4, VP, maxP]
        maxP = int(max(PA.max(), PB.max(), 1))
        mats = np.full((4, VP, maxP), ZID, np.int16)
        mask = np.arange(D)[None, :] < deg[bi][:, None]
        u_e = np.repeat(np.arange(V), D)[mask.ravel()]
        n_e = nbr_idx[bi].ravel()[mask.ravel()]
        k_e = q[n_e].astype(np.int64)
        key = u_e.astype(np.int64) * 4 + k_e
        o2 = np.argsort(key, kind="stable")
        u_s, n_s, k_s = u_e[o2], n_e[o2], k_e[o2]
        key_s = key[o2]
        first = np.searchsorted(key_s, key_s, side="left")
        rank = np.arange(len(key_s)) - first
        mats[k_s, u_s, rank] = qlocal[n_s].astype(np.int16)
        assert mats.max() <= ZID and mats.min() >= 0

        tgt_mat = order.reshape(NB, BLK)
        wrapped_parts = []
        for pair, groups in chunks:
            bs = np.concatenate([[b_] * n for b_, n in groups]).astype(np.int64)
            ps = np.concatenate([np.arange(n) for _, n in groups]).astype(np.int64)
            mlow, mhigh = (mats[0], mats[1]) if pair == "A" else (mats[2], mats[3])
            tgts = tgt_mat[bs]                                 # [nplanes, BLK]
            low = mlow[tgts, ps[:, None]].reshape(-1)
            high = mhigh[tgts, ps[:, None]].reshape(-1)
            ncols = len(bs) * BLK
            wl = low.reshape(ncols // 16, 16).T
            wh = high.reshape(ncols // 16, 16).T
            wrap = np.concatenate([np.tile(wl, (4, 1)), np.tile(wh, (4, 1))], axis=0)
            wrapped_parts.append(wrap.astype(np.int16))
        idxs = np.ascontiguousarray(np.concatenate(wrapped_parts, axis=1))

        degf = np.maximum(deg[bi], 1).astype(np.float32)
        invd_vp = np.ones(VP, np.float32)
        invd_vp[:V] = 1.0 / degf
        deg_vp = np.ones(VP, np.float32)
        deg_vp[:V] = degf
        xs_vp = np.zeros((C, VP), np.float32)
        valid = order < V
        xs_vp[:, valid] = xb[:, order[valid]]
        # self term pre-scaled by deg: psum = deg*W0*x + sum W1*x_nbr, then *1/deg
        xs_vp *= deg_vp[order][None, :]
        combo = np.concatenate(
            [xs_vp, np.tile(invd_vp[order][None, :], (C, 1))], axis=0
        ).astype(ml_dtypes.bfloat16)

        in_maps.append(
            {
                "tbl": tblf,
                "combo": combo,
                "idxs": idxs,
                "w1cat": w1cat.astype(ml_dtypes.bfloat16),
                "w1f": w1cat,
                "w0t": w0.astype(ml_dtypes.bfloat16),
                "bias": bvec,
            }
        )
    orders = [pc[1] for pc in per_core]
    return (tuple(PA.tolist()), tuple(PB.tolist())), in_maps, orders


_CACHE = {}
TRACE = False
LAST_RESULT = None


def _get_nc(PA, PB):
    key = (PA, PB)
    if key not in _CACHE:
        _CACHE[key] = build_nc(np.array(PA), np.array(PB))
    return _CACHE[key]


def kernel(x, nbr_idx, deg, W, b):
    global LAST_RESULT
    x = np.asarray(x, np.float32)
    nbr_idx = np.asarray(nbr_idx, np.int32)
    deg = np.asarray(deg, np.int32)
    W = np.asarray(W, np.float32)
    b = np.asarray(b, np.float32)
    (PA, PB), in_maps, orders = _profile_and_maps(x, nbr_idx, deg, W, b)
    nc = _get_nc(PA, PB)
    try:
        res = run_bass_kernel_spmd(nc, in_maps, list(range(len(in_maps))), trace=TRACE)
    except ModuleNotFoundError:
        res = run_bass_kernel_spmd(nc, in_maps, list(range(len(in_maps))), trace=False)
    LAST_RESULT = res
    outs = []
    for bi, r in enumerate(res.results):
        order = orders[bi]
        valid = order < V
        ob = np.empty((O, V), np.float32)
        ob[:, order[valid]] = r["out"][:, valid]
        outs.append(ob)
    out = np.stack(outs, axis=0)
    return out[..., None].astype(np.float32)
